# revision 3
# baseline (speedup 1.0000x reference)
"""Trainium2 Bass kernel for single-head attention (no V projection).

Reference computation (per batch b):
    q = x @ Wq ; k = x @ Wk
    scores = q @ k.T / sqrt(64)
    out = softmax(scores, axis=-1) @ x

Key algebraic rewrite: scores = (x Wq)(x Wk)^T / 8 = x A x^T with
A = Wq Wk^T / 8 precomputed on the host. Each core then projects only
its OWN query rows (y = x_q @ A) and uses x^T (already resident in
SBUF for the projection) directly as the scores lhsT — the entire k
projection (2048x1024x1024 per core, ~74us of PE time including the
cross-core redundancy) disappears. Per-core PE work drops from
15.0 GF to 10.75 GF with no collectives and identical statistics
(A ~ N(0,1/D) like Wq, y ~ N(0,1) like q).

Shapes: x [4, 2048, 1024], Wq/Wk [1024, 1024] -> out [4, 2048, 1024] fp32.

Sharding: 8 cores, core c handles batch b=c//2, query-row half h=c%2.
Each core receives its batch's x rolled so its 1024 query rows come
first (attention is permutation-invariant over keys). The projection
and scores contract over the model dim, which must sit on SBUF
partitions, so x^T is built on the host (trn2 has no fp32
DMA-transpose).

On-chip dataflow (all matmuls contract over the partition dim):
    yT  [e, s]  = A.T @ x_q.T     (lhsT=A tile, rhs=xT)
    scoresT [t, s] = xT-chunks.T @ yT   (lhsT=xT, rhs=yT)
    expT = Exp(scoresT)           (ScalarE eviction from PSUM)
    sumexp [s, 2] = expT.T @ ones (N=2 matmuls; fp32r needs N>=2)
    out [s, d] = expT.T @ x       (lhsT=expT, rhs=x natural)
    out scaled by 1/sumexp on the DVE during PSUM->SBUF eviction.

DMA head: one dma_start lands on one of 16 hw queues (~44GB/s each;
transfers only begin ~8.7us in — fixed pipeline startup), so the
phase-B streams are pre-chunked ON THE HOST into fully-dense 128-256KB
blocks (max packet size, no strided segments) and issued in exactly
consumption order: A's e=0 quarter-chunks, then x^T for the query rows
(s<1024) in 256-col blocks, then the rest of A, then x^T for the
remaining keys, then x natural. The first yT group is split into two
N=256 halves so it only needs ~1.5MB before the PE can leave warmup.

A burst of warmup matmuls on uninitialized SBUF runs immediately (no
DMA dependency; results land in a never-read PSUM bank) so the PE's
HAM clock-gate ramps to 2.4 GHz before the first real matmul issues
(the first ~13 matmuls run at 1.2 GHz otherwise).

Matmul operands live in SBUF as float32r (fp32 bits; the PE truncates
to FP22 on read — 1 cycle/row at free-dim>=256 vs 4 for fp32; measured
end-to-end rel err ~5e-4). The BIR verifier requires every producer of
an fp32r matmul operand to write fp32r-typed data, so DMA'd tiles use
fp32r DRAM params and PSUM evictions write to fp32r tiles.

Softmax skips the max-subtraction: scores have std ~4 and |max| < ~25,
so exp stays comfortably inside fp32 range and the result is
mathematically identical to jax.nn.softmax.
"""

from contextlib import ExitStack

import numpy as np

import concourse.bacc as bacc
import concourse.tile as tile
from concourse import mybir
from concourse.bass_utils import run_bass_kernel_spmd

F32 = mybir.dt.float32
F32R = mybir.dt.float32r
AFT = mybir.ActivationFunctionType

P = 128      # partitions
S = 2048     # keys (t) per batch
SQ = 1024    # query rows per core
D = 1024     # model dim
NT = S // P  # 16 t-chunks
ND = D // P  # 8 d/e-chunks
SB = 512     # query-block width in phase C
NSB = SQ // SB
N_WARMUP = 12   # N=512 matmuls on uninitialized SBUF (no DMA dependency,
                # results never read): the clock ramp runs ~795+11*427ns,
                # ending ~12.5us in, right as the first projection
                # inputs land.

B_FULL, S_FULL, D_FULL = 4, 2048, 1024
N_CORES = 8

_NC_CACHE = None
LAST_RESULT = None  # BassKernelResults of the most recent kernel() call
TRACE = False      # set by test.py to capture an NTFF profile
TRACE_DIR = None


def _r(ap):
    return ap.bitcast(F32R)


def _build_nc():
    global _NC_CACHE
    if _NC_CACHE is not None:
        return _NC_CACHE

    nc = bacc.Bacc("TRN2")
    # x natural, for the attn @ x stage (dense 512KB row-blocks)
    x = nc.declare_dram_parameter("x", [S, D], F32R, isOutput=False)
    # x^T s<1024 (query rows), 32 dense [128,256] blocks, h-major:
    #   block h*8+d = x^T[d*128:(d+1)*128, h*256:(h+1)*256]
    xth = nc.declare_dram_parameter("xth", [32 * P, 256], F32R, isOutput=False)
    # x^T s>=1024, 16 dense [128,512] blocks:
    #   block st*8+d = x^T[d*128:(d+1)*128, 1024+st*512 : 1024+(st+1)*512]
    xth2 = nc.declare_dram_parameter("xth2", [16 * P, 512], F32R, isOutput=False)
    # A = Wq @ Wk.T / 8 in e-major SBUF layout
    #   (walayout[p, e*D + dd*P + j] = A[dd*P+p, e*P+j]), pre-chunked:
    # wah: e=0 block as 4 dense [128,256] chunks
    wah = nc.declare_dram_parameter("wah", [4 * P, 256], F32R, isOutput=False)
    # wa: e=1..7 blocks as 14 dense [128,512] chunks
    wa = nc.declare_dram_parameter("wa", [14 * P, 512], F32R, isOutput=False)
    ones = nc.declare_dram_parameter("ones", [P, 2], F32R, isOutput=False)
    out = nc.declare_dram_parameter("out", [SQ, D], F32, isOutput=True)

    with tile.TileContext(nc) as tc, ExitStack() as ctx:
        singles = ctx.enter_context(tc.tile_pool(name="singles", bufs=1))
        ot = singles.tile([P, 2], F32R)

        persist = ctx.enter_context(tc.tile_pool(name="persist", bufs=1))
        # d-chunk d at [:, d*S : (d+1)*S] (free axis = s over all 2048
        # keys); doubles as the scores lhsT in phase C.
        xT = persist.tile([P, ND * S], F32R, tag="xT")
        # e-chunk e at [:, e*SQ : (e+1)*SQ] (free axis = s query)
        yT = persist.tile([P, ND * SQ], F32R, tag="yT")
        # x natural: t-chunk t at [:, t*D : (t+1)*D]
        xc = persist.tile([P, NT * D], F32R, tag="xc")

        # ---------- phase B: load xt/A, project y ----------
        with tc.tile_pool(name="wA", bufs=1) as wA_pool, \
             tc.tile_pool(name="ps_w", bufs=1, space="PSUM") as ps_w_pool, \
             tc.tile_pool(name="ps_pj", bufs=4, space="PSUM") as ps_pj:
            wt = wA_pool.tile([P, ND * D], F32R, tag="w")

            # PE warmup on uninitialized yT contents (yT's first real
            # write is a DVE eviction later, so no dependency and no
            # delay); garbage results land in a scratch PSUM bank that
            # is never read.
            ps_w = ps_w_pool.tile([P, 512], F32)
            for i in range(N_WARMUP):
                nc.tensor.matmul(ps_w[:], yT[:, 0:P], yT[:, SQ:SQ + 512],
                                 start=(i == 0), stop=(i == N_WARMUP - 1))

            # DMA issue order = queue assignment = arrival order.
            for c in range(4):
                nc.sync.dma_start(
                    out=wt[:, c * 256:(c + 1) * 256],
                    in_=wah[c * P:(c + 1) * P, :],
                )
            for h in range(4):
                for d in range(ND):
                    blk = h * ND + d
                    nc.sync.dma_start(
                        out=xT[:, d * S + h * 256: d * S + (h + 1) * 256],
                        in_=xth[blk * P:(blk + 1) * P, :],
                    )
            for k in range(14):
                e, c = 1 + k // 2, k % 2
                nc.sync.dma_start(
                    out=wt[:, e * D + c * 512: e * D + (c + 1) * 512],
                    in_=wa[k * P:(k + 1) * P, :],
                )
            for st in range(2):
                for d in range(ND):
                    blk = st * ND + d
                    nc.sync.dma_start(
                        out=xT[:, d * S + 1024 + st * 512:
                               d * S + 1024 + (st + 1) * 512],
                        in_=xth2[blk * P:(blk + 1) * P, :],
                    )
            for t in range(NT):
                nc.sync.dma_start(
                    out=xc[:, t * D:(t + 1) * D], in_=x[t * P:(t + 1) * P, :]
                )
            nc.sync.dma_start(out=ot[:], in_=ones[:])

            # yT projection; e ascending matches A's DMA arrival order.
            # 512-wide stripes (narrower are LDWEIGHTS-bound) except the
            # very first group, split in two so the PE can start on
            # ~1.5MB of input instead of ~2.5MB.
            for e in range(ND):
                sh_parts = ([(0, 256), (256, 256), (512, 512)] if e == 0
                            else [(0, 512), (512, 512)])
                for s0, w in sh_parts:
                    ps = ps_pj.tile([P, w], F32)
                    for dd in range(ND):
                        nc.tensor.matmul(
                            ps[:],
                            wt[:, e * D + dd * P: e * D + (dd + 1) * P],
                            xT[:, dd * S + s0: dd * S + s0 + w],
                            start=(dd == 0), stop=(dd == ND - 1),
                        )
                    nc.vector.tensor_copy(
                        yT[:, e * SQ + s0: e * SQ + s0 + w], _r(ps[:]),
                    )

        # ---------- phase C: scores -> softmax -> attn @ x ----------
        with tc.tile_pool(name="exp", bufs=1) as exp_pool, \
             tc.tile_pool(name="outp", bufs=4) as out_pool, \
             tc.tile_pool(name="recip", bufs=4) as recip_pool, \
             tc.tile_pool(name="partial", bufs=2) as partial_pool, \
             tc.tile_pool(name="ps_sc", bufs=4, space="PSUM") as ps_sc, \
             tc.tile_pool(name="ps_av", bufs=3, space="PSUM") as ps_av, \
             tc.tile_pool(name="ps_sum", bufs=1, space="PSUM") as ps_sum:
            for blk in range(NSB):
                # t-chunk t at [:, t*SB : (t+1)*SB] (free axis = s within blk)
                expT = exp_pool.tile([P, NT * SB], F32R, tag="expT")
                # Softmax denominator: the 16-chunk accumulation runs on
                # the (otherwise idle) DVE as a chain of adds interleaved
                # with the scores loop; the PE then only does one N=2
                # partition-reduce matmul per s-chunk instead of 16
                # LDWEIGHTS-bound ones each.
                partial = partial_pool.tile([P, SB], F32R, tag="partial")
                for t in range(NT):
                    ps = ps_sc.tile([P, SB], F32)
                    for dd in range(ND):
                        nc.tensor.matmul(
                            ps[:],
                            xT[:, dd * S + t * P: dd * S + (t + 1) * P],
                            yT[:, dd * SQ + blk * SB: dd * SQ + (blk + 1) * SB],
                            start=(dd == 0), stop=(dd == ND - 1),
                        )
                    nc.scalar.activation(expT[:, t * SB:(t + 1) * SB], ps[:], AFT.Exp)
                    if t == 1:
                        nc.vector.tensor_add(
                            partial[:], expT[:, 0:SB], expT[:, SB:2 * SB])
                    elif t >= 2:
                        nc.vector.tensor_add(
                            partial[:], partial[:],
                            expT[:, t * SB:(t + 1) * SB])

                for ss in range(SB // P):
                    pss = ps_sum.tile([P, 2], F32)
                    nc.tensor.matmul(
                        pss[:], partial[:, ss * P:(ss + 1) * P], ot[:],
                        start=True, stop=True,
                    )
                    rec = recip_pool.tile([P, 1], F32, tag="rec")
                    nc.vector.reciprocal(rec[:], pss[:, 0:1])

                    for dh in range(2):
                        psa = ps_av.tile([P, 512], F32)
                        for t in range(NT):
                            nc.tensor.matmul(
                                psa[:],
                                expT[:, t * SB + ss * P: t * SB + (ss + 1) * P],
                                xc[:, t * D + dh * 512: t * D + dh * 512 + 512],
                                start=(t == 0), stop=(t == NT - 1),
                            )
                        ob = out_pool.tile([P, 512], F32, tag="ob")
                        nc.vector.tensor_scalar_mul(ob[:], psa[:], rec[:, 0:1])
                        row0 = blk * SB + ss * P
                        # two row-halves on two hw queues to shrink the
                        # final-store tail
                        for rh in range(2):
                            nc.sync.dma_start(
                                out=out[row0 + rh * 64:row0 + (rh + 1) * 64,
                                        dh * 512:dh * 512 + 512],
                                in_=ob[rh * 64:(rh + 1) * 64, :],
                            )

    nc.finalize()
    _NC_CACHE = nc
    return nc


def kernel(inputs, Wq, Wk):
    global LAST_RESULT
    x = np.asarray(inputs, dtype=np.float32)
    assert x.shape == (B_FULL, S_FULL, D_FULL)
    A = (np.asarray(Wq, dtype=np.float32) @ np.asarray(Wk, dtype=np.float32).T
         ) * np.float32(0.125)
    # walayout[p, e*D + dd*P + j] = A[dd*P + p, e*P + j]
    walayout = A.reshape(ND, P, ND, P).transpose(1, 2, 0, 3).reshape(P, ND * D)
    wah = np.ascontiguousarray(
        walayout[:, 0:D].reshape(P, 4, 256).transpose(1, 0, 2).reshape(4 * P, 256))
    wa = np.ascontiguousarray(
        walayout[:, D:].reshape(P, 14, 512).transpose(1, 0, 2).reshape(14 * P, 512))
    ones = np.ones((P, 2), dtype=np.float32)

    nc = _build_nc()

    in_maps = []
    for c in range(N_CORES):
        b, h = c // 2, c % 2
        xb = x[b]
        if h:
            xb = np.concatenate([xb[SQ:], xb[:SQ]], axis=0)
        xbt = xb.T  # [D, S]
        xth = np.ascontiguousarray(
            xbt[:, :SQ].reshape(ND, P, 4, 256).transpose(2, 0, 1, 3)
            .reshape(32 * P, 256))
        xth2 = np.ascontiguousarray(
            xbt[:, SQ:].reshape(ND, P, 2, 512).transpose(2, 0, 1, 3)
            .reshape(16 * P, 512))
        in_maps.append({
            "x": np.ascontiguousarray(xb),
            "xth": xth,
            "xth2": xth2,
            "wah": wah,
            "wa": wa,
            "ones": ones,
        })

    kwargs = {}
    if TRACE:
        kwargs = {"trace": True, "tmpdir": TRACE_DIR}
    res = run_bass_kernel_spmd(nc, in_maps, list(range(N_CORES)), **kwargs)
    LAST_RESULT = res

    full = np.empty((B_FULL, S_FULL, D_FULL), dtype=np.float32)
    for c in range(N_CORES):
        b, h = c // 2, c % 2
        full[b, h * SQ:(h + 1) * SQ, :] = res.results[c]["out"]
    return full


# revision 5
# speedup vs baseline: 1.0607x; 1.0607x over previous
"""Trainium2 Bass kernel for single-head attention (no V projection).

Reference computation (per batch b):
    q = x @ Wq ; k = x @ Wk
    scores = q @ k.T / sqrt(64)
    out = softmax(scores, axis=-1) @ x

Key algebraic rewrite: scores = (x Wq)(x Wk)^T / 8 = x A x^T with
A = Wq Wk^T / 8 precomputed on the host. Each core then projects only
its OWN query rows (y = x_q @ A) and uses x^T (already resident in
SBUF for the projection) directly as the scores lhsT — the entire k
projection (2048x1024x1024 per core, ~74us of PE time including the
cross-core redundancy) disappears. Per-core PE work drops from
15.0 GF to 10.75 GF with no collectives and identical statistics
(A ~ N(0,1/D) like Wq, y ~ N(0,1) like q).

Shapes: x [4, 2048, 1024], Wq/Wk [1024, 1024] -> out [4, 2048, 1024] fp32.

Sharding: 8 cores, core c handles batch b=c//2, query-row half h=c%2.
Each core receives its batch's x rolled so its 1024 query rows come
first (attention is permutation-invariant over keys), plus the same x
pre-transposed on the host (xt) — the PE contracts over the partition
dim, so the projection/scores need x with the model dim on partitions,
and trn2 has no fp32 DMA-transpose.

On-chip dataflow (all matmuls contract over the partition dim):
    yT  [e, s]  = A.T @ x_q.T     (lhsT=A tile, rhs=xT)
    scoresT [t, s] = xT-chunks.T @ yT   (lhsT=xT, rhs=yT)
    expT = Exp(scoresT)           (ScalarE eviction from PSUM)
    sumexp [s, 2] = expT.T @ ones (N=2 matmuls; fp32r needs N>=2)
    out [s, d] = expT.T @ x       (lhsT=expT, rhs=x natural)
    out scaled by 1/sumexp on the DVE during PSUM->SBUF eviction.

DMA: one dma_start lands on one of 16 hw queues; per-queue rate is
~21.5GB/s sustained (rate halves below 2KB SBUF rows — keep 512-col
chunks) and transfers only begin ~8.7us in (fixed pipeline startup).
A is pre-arranged on the host into the e-major SBUF layout
(walayout[p, e*D + dd*P + j] = A[dd*P+p, e*P+j]) and every stream is
issued in consumption order in <=256KB chunks so no single queue gates
progress: A e=0, x^T query halves, A e=1..7, x^T key halves, x
natural, with the output stores row-split across two queues to shrink
the final-store tail.

A burst of warmup matmuls on uninitialized SBUF runs immediately (no
DMA dependency; results land in a never-read PSUM bank) so the PE's
HAM clock-gate ramps to 2.4 GHz before the first real matmul issues
(the first ~13 matmuls run at 1.2 GHz otherwise) and the PE never
idles >3.4us (which would drop the clock again).

Matmul operands live in SBUF as float32r (fp32 bits; the PE truncates
to FP22 on read — 1 cycle/row at free-dim>=256 vs 4 for fp32; measured
end-to-end rel err ~5e-4). The BIR verifier requires every producer of
an fp32r matmul operand to write fp32r-typed data, so DMA'd tiles use
fp32r DRAM params and PSUM evictions write to fp32r tiles.

Softmax skips the max-subtraction: scores have std ~4 and |max| < ~25,
so exp stays comfortably inside fp32 range and the result is
mathematically identical to jax.nn.softmax.
"""

from contextlib import ExitStack

import numpy as np

import concourse.bacc as bacc
import concourse.tile as tile
from concourse import mybir
from concourse.bass_utils import run_bass_kernel_spmd

F32 = mybir.dt.float32
F32R = mybir.dt.float32r
AFT = mybir.ActivationFunctionType

P = 128      # partitions
S = 2048     # keys (t) per batch
SQ = 1024    # query rows per core
D = 1024     # model dim
NT = S // P  # 16 t-chunks
ND = D // P  # 8 d/e-chunks
SB = 512     # query-block width in phase C
NSB = SQ // SB
N_WARMUP = 32   # N=512 matmuls on uninitialized SBUF (no DMA dependency,
                # results never read): ~13 ramp the clock (795+427ns
                # each), the rest run at 227ns, ending ~17.5us in —
                # just before the first projection inputs land (~19.5us,
                # bound by ~2.5MB critical DMA over the shared fabric).

B_FULL, S_FULL, D_FULL = 4, 2048, 1024
N_CORES = 8

_NC_CACHE = None
LAST_RESULT = None  # BassKernelResults of the most recent kernel() call
TRACE = False      # set by test.py to capture an NTFF profile
TRACE_DIR = None


def _r(ap):
    return ap.bitcast(F32R)


def _build_nc():
    global _NC_CACHE
    if _NC_CACHE is not None:
        return _NC_CACHE

    nc = bacc.Bacc("TRN2")
    x = nc.declare_dram_parameter("x", [S, D], F32R, isOutput=False)
    xt = nc.declare_dram_parameter("xt", [D, S], F32R, isOutput=False)
    # A = Wq @ Wk.T / 8 in the e-major SBUF layout (see module docstring)
    wa = nc.declare_dram_parameter("wa", [P, ND * D], F32R, isOutput=False)
    ones = nc.declare_dram_parameter("ones", [P, 2], F32R, isOutput=False)
    out = nc.declare_dram_parameter("out", [SQ, D], F32, isOutput=True)

    with tile.TileContext(nc) as tc, ExitStack() as ctx:
        singles = ctx.enter_context(tc.tile_pool(name="singles", bufs=1))
        ot = singles.tile([P, 2], F32R)

        persist = ctx.enter_context(tc.tile_pool(name="persist", bufs=1))
        # d-chunk d at [:, d*S : (d+1)*S] (free axis = s over all 2048
        # keys); doubles as the scores lhsT in phase C.
        xT = persist.tile([P, ND * S], F32R, tag="xT")
        # e-chunk e at [:, e*SQ : (e+1)*SQ] (free axis = s query)
        yT = persist.tile([P, ND * SQ], F32R, tag="yT")
        # x natural: t-chunk t at [:, t*D : (t+1)*D]
        xc = persist.tile([P, NT * D], F32R, tag="xc")

        # ---------- phase B: load xt/A, project y ----------
        with tc.tile_pool(name="wA", bufs=1) as wA_pool, \
             tc.tile_pool(name="ps_w", bufs=1, space="PSUM") as ps_w_pool, \
             tc.tile_pool(name="ps_pj", bufs=4, space="PSUM") as ps_pj:
            wt = wA_pool.tile([P, ND * D], F32R, tag="w")

            # PE warmup on uninitialized yT contents (yT's first real
            # write is a DVE eviction later, so no dependency and no
            # delay); garbage results land in a scratch PSUM bank that
            # is never read.
            ps_w = ps_w_pool.tile([P, 512], F32)
            for i in range(N_WARMUP):
                nc.tensor.matmul(ps_w[:], yT[:, 0:P], yT[:, SQ:SQ + 512],
                                 start=(i == 0), stop=(i == N_WARMUP - 1))

            # DMA issue order = queue assignment (round-robin) =
            # arrival order. Everything in <=256KB chunks with 2KB rows.
            for c in range(2):
                nc.sync.dma_start(
                    out=wt[:, c * 512:(c + 1) * 512],
                    in_=wa[:, c * 512:(c + 1) * 512],
                )
            for st in range(2):
                for d in range(ND):
                    nc.sync.dma_start(
                        out=xT[:, d * S + st * 512: d * S + (st + 1) * 512],
                        in_=xt[d * P:(d + 1) * P, st * 512:(st + 1) * 512],
                    )
            for k in range(2, 2 * ND):
                nc.sync.dma_start(
                    out=wt[:, k * 512:(k + 1) * 512],
                    in_=wa[:, k * 512:(k + 1) * 512],
                )
            for st in range(2, 4):
                for d in range(ND):
                    nc.sync.dma_start(
                        out=xT[:, d * S + st * 512: d * S + (st + 1) * 512],
                        in_=xt[d * P:(d + 1) * P, st * 512:(st + 1) * 512],
                    )
            for t in range(NT):
                nc.sync.dma_start(
                    out=xc[:, t * D:(t + 1) * D], in_=x[t * P:(t + 1) * P, :]
                )
            nc.sync.dma_start(out=ot[:], in_=ones[:])

            # yT projection; e ascending matches A's DMA arrival order,
            # 512-wide stripes only — narrower ones are LDWEIGHTS-bound
            for e in range(ND):
                for sh in range(SQ // 512):
                    ps = ps_pj.tile([P, 512], F32)
                    for dd in range(ND):
                        nc.tensor.matmul(
                            ps[:],
                            wt[:, e * D + dd * P: e * D + (dd + 1) * P],
                            xT[:, dd * S + sh * 512: dd * S + sh * 512 + 512],
                            start=(dd == 0), stop=(dd == ND - 1),
                        )
                    nc.vector.tensor_copy(
                        yT[:, e * SQ + sh * 512: e * SQ + sh * 512 + 512],
                        _r(ps[:]),
                    )

        # ---------- phase C: scores -> softmax -> attn @ x ----------
        with tc.tile_pool(name="exp", bufs=1) as exp_pool, \
             tc.tile_pool(name="outp", bufs=4) as out_pool, \
             tc.tile_pool(name="recip", bufs=4) as recip_pool, \
             tc.tile_pool(name="partial", bufs=2) as partial_pool, \
             tc.tile_pool(name="ps_sc", bufs=3, space="PSUM") as ps_sc, \
             tc.tile_pool(name="ps_av", bufs=2, space="PSUM") as ps_av, \
             tc.tile_pool(name="ps_sum", bufs=1, space="PSUM") as ps_sum:
            for blk in range(NSB):
                # t-chunk t at [:, t*SB : (t+1)*SB] (free axis = s within blk)
                expT = exp_pool.tile([P, NT * SB], F32R, tag="expT")
                # Softmax denominator: the 16-chunk accumulation runs on
                # the (otherwise idle) DVE as a chain of adds interleaved
                # with the scores loop; the PE then only does one N=2
                # partition-reduce matmul per s-chunk instead of 16
                # LDWEIGHTS-bound ones each.
                partial = partial_pool.tile([P, SB], F32R, tag="partial")
                for t in range(NT):
                    ps = ps_sc.tile([P, SB], F32)
                    for dd in range(ND):
                        nc.tensor.matmul(
                            ps[:],
                            xT[:, dd * S + t * P: dd * S + (t + 1) * P],
                            yT[:, dd * SQ + blk * SB: dd * SQ + (blk + 1) * SB],
                            start=(dd == 0), stop=(dd == ND - 1),
                        )
                    nc.scalar.activation(expT[:, t * SB:(t + 1) * SB], ps[:], AFT.Exp)
                    if t == 1:
                        nc.vector.tensor_add(
                            partial[:], expT[:, 0:SB], expT[:, SB:2 * SB])
                    elif t >= 2:
                        nc.vector.tensor_add(
                            partial[:], partial[:],
                            expT[:, t * SB:(t + 1) * SB])

                for ss in range(SB // P):
                    # The first attn half-group goes on the PE ahead of
                    # the tiny denominator matmul, so the PE never waits
                    # on the DVE's partial-sum chain; the reciprocal is
                    # ready by the time the dh=0 eviction needs it.
                    psa0 = ps_av.tile([P, 512], F32)
                    for t in range(NT):
                        nc.tensor.matmul(
                            psa0[:],
                            expT[:, t * SB + ss * P: t * SB + (ss + 1) * P],
                            xc[:, t * D: t * D + 512],
                            start=(t == 0), stop=(t == NT - 1),
                        )
                    pss = ps_sum.tile([P, 2], F32)
                    nc.tensor.matmul(
                        pss[:], partial[:, ss * P:(ss + 1) * P], ot[:],
                        start=True, stop=True,
                    )
                    rec = recip_pool.tile([P, 1], F32, tag="rec")
                    nc.vector.reciprocal(rec[:], pss[:, 0:1])
                    psa1 = ps_av.tile([P, 512], F32)
                    for t in range(NT):
                        nc.tensor.matmul(
                            psa1[:],
                            expT[:, t * SB + ss * P: t * SB + (ss + 1) * P],
                            xc[:, t * D + 512: t * D + 1024],
                            start=(t == 0), stop=(t == NT - 1),
                        )
                    row0 = blk * SB + ss * P
                    for dh, psa in ((0, psa0), (1, psa1)):
                        ob = out_pool.tile([P, 512], F32, tag="ob")
                        nc.vector.tensor_scalar_mul(ob[:], psa[:], rec[:, 0:1])
                        # two row-halves on two hw queues to shrink the
                        # final-store tail
                        for rh in range(2):
                            nc.sync.dma_start(
                                out=out[row0 + rh * 64:row0 + (rh + 1) * 64,
                                        dh * 512:dh * 512 + 512],
                                in_=ob[rh * 64:(rh + 1) * 64, :],
                            )

    nc.finalize()
    _NC_CACHE = nc
    return nc


def kernel(inputs, Wq, Wk):
    global LAST_RESULT
    x = np.asarray(inputs, dtype=np.float32)
    assert x.shape == (B_FULL, S_FULL, D_FULL)
    A = (np.asarray(Wq, dtype=np.float32) @ np.asarray(Wk, dtype=np.float32).T
         ) * np.float32(0.125)
    # walayout[p, e*D + dd*P + j] = A[dd*P + p, e*P + j]
    wa = np.ascontiguousarray(
        A.reshape(ND, P, ND, P).transpose(1, 2, 0, 3).reshape(P, ND * D))
    ones = np.ones((P, 2), dtype=np.float32)

    nc = _build_nc()

    in_maps = []
    for c in range(N_CORES):
        b, h = c // 2, c % 2
        xb = x[b]
        if h:
            xb = np.concatenate([xb[SQ:], xb[:SQ]], axis=0)
        in_maps.append({
            "x": np.ascontiguousarray(xb),
            "xt": np.ascontiguousarray(xb.T),
            "wa": wa,
            "ones": ones,
        })

    kwargs = {}
    if TRACE:
        kwargs = {"trace": True, "tmpdir": TRACE_DIR}
    res = run_bass_kernel_spmd(nc, in_maps, list(range(N_CORES)), **kwargs)
    LAST_RESULT = res

    full = np.empty((B_FULL, S_FULL, D_FULL), dtype=np.float32)
    for c in range(N_CORES):
        b, h = c // 2, c % 2
        full[b, h * SQ:(h + 1) * SQ, :] = res.results[c]["out"]
    return full


# revision 7
# speedup vs baseline: 1.1072x; 1.0438x over previous
"""Trainium2 Bass kernel for single-head attention (no V projection).

Reference computation (per batch b):
    q = x @ Wq ; k = x @ Wk
    scores = q @ k.T / sqrt(64)
    out = softmax(scores, axis=-1) @ x

Key algebraic rewrite: scores = (x Wq)(x Wk)^T / 8 = x A x^T with
A = Wq Wk^T / 8 precomputed on the host. Each core then projects only
its OWN query rows (y = x_q @ A) and uses x^T (already resident in
SBUF for the projection) directly as the scores lhsT — the entire k
projection disappears. Per-core PE work drops from 15.0 GF to 10.75 GF
with no collectives and identical statistics (A ~ N(0,1/D) like Wq,
y ~ N(0,1) like q).

Shapes: x [4, 2048, 1024], Wq/Wk [1024, 1024] -> out [4, 2048, 1024] fp32.

Sharding: 8 cores, core c handles batch b=c//2, query-row half h=c%2.
Each core receives its batch's x rolled so its 1024 query rows come
first (attention is permutation-invariant over keys), plus the same x
pre-transposed on the host (xt) — the PE contracts over the partition
dim, and trn2 has no DMA-transpose.

All matmul operands are bf16 (host-rounded): the PE streams bf16 at
1 cycle/row like fp32r, but every DMA stream halves in bytes AND in
row count (the per-queue DMA bottleneck is ~95ns per >=2KB row), and
the whole working set (x^T 8MB, y^T 2MB, x 4MB, A 2MB, exp 2MB bf16)
stays SBUF-resident together. Accumulation is fp32 in PSUM and the
softmax denominator accumulates fp32 on the DVE, so the only precision
loss is input/intermediate rounding: measured end-to-end rel err ~2e-3
against the fp32 reference (budget 2e-2). expT must be bf16 (not fp16)
for range: scores reach ~25 and e^25 overflows fp16.

On-chip dataflow (all matmuls contract over the partition dim):
    yT  [e, s]  = A.T @ x_q.T     (lhsT=A tile, rhs=xT)
    scoresT [t, s] = xT-chunks.T @ yT   (lhsT=xT, rhs=yT)
    expT = Exp(scoresT)           (ScalarE eviction from PSUM)
    sumexp [s, 2] = partial @ ones (fp32 N=2 matmul; partial = DVE
                                   fp32 chain-sum of expT t-chunks)
    out [s, d] = expT.T @ x       (lhsT=expT, rhs=x natural)
    out scaled by 1/sumexp on the DVE during PSUM->SBUF eviction (fp32).

Schedule: the query-half projections are split around score block 0 —
    warmup | yT(s<512) | scores/attn blk0 | yT(s>=512) | scores/attn blk1
so only A + x^T(s<512) gate the first real matmul (~1.5MB of DMA) and
the second half-projection runs in the DMA-quiet window. The PE never
idles once started; engine deps (DVE evictions, ScalarE exp, fp32
denominator) all hide under adjacent matmul groups.

DMA: one dma_start lands on one of 16 hw queues (round-robin by issue
order); transfers begin ~8.7us in (fixed pipeline startup) and each
queue moves one >=2KB row per ~95ns. Streams are issued in consumption
order, chunked [64,1024] (A's first block [32,1024]) so no single
queue gates progress. A is pre-arranged on the host into the e-major
SBUF layout (walayout[p, e*D + dd*P + j] = A[dd*P+p, e*P+j]).

A burst of warmup matmuls on uninitialized SBUF runs immediately (no
DMA dependency; results land in a never-read PSUM bank) so the PE's
HAM clock-gate ramps to 2.4 GHz before the first real matmul issues
(the first ~13 matmuls run at 1.2 GHz otherwise).

Softmax skips the max-subtraction: scores have std ~4 and |max| < ~25,
so exp stays comfortably inside fp32/bf16 range and the result is
mathematically identical to jax.nn.softmax.
"""

from contextlib import ExitStack

import ml_dtypes
import numpy as np

import concourse.bacc as bacc
import concourse.tile as tile
from concourse import mybir
from concourse.bass_utils import run_bass_kernel_spmd

F32 = mybir.dt.float32
BF16 = mybir.dt.bfloat16
AFT = mybir.ActivationFunctionType

P = 128      # partitions
S = 2048     # keys (t) per batch
SQ = 1024    # query rows per core
D = 1024     # model dim
NT = S // P  # 16 t-chunks
ND = D // P  # 8 d/e-chunks
SB = 512     # query-block width in phase C
NSB = SQ // SB
N_WARMUP = 30   # N=512 matmuls on uninitialized SBUF (no DMA dependency,
                # results never read): ~13 ramp the clock (795+427ns
                # each), the rest run at 227ns, ending ~17us in — right
                # as the first projection inputs land.

B_FULL, S_FULL, D_FULL = 4, 2048, 1024
N_CORES = 8

_NC_CACHE = None
LAST_RESULT = None  # BassKernelResults of the most recent kernel() call
TRACE = False      # set by test.py to capture an NTFF profile
TRACE_DIR = None


def _build_nc():
    global _NC_CACHE
    if _NC_CACHE is not None:
        return _NC_CACHE

    nc = bacc.Bacc("TRN2")
    x = nc.declare_dram_parameter("x", [S, D], BF16, isOutput=False)
    xt = nc.declare_dram_parameter("xt", [D, S], BF16, isOutput=False)
    # A = Wq @ Wk.T / 8 in the e-major SBUF layout (see module docstring)
    wa = nc.declare_dram_parameter("wa", [P, ND * D], BF16, isOutput=False)
    ones = nc.declare_dram_parameter("ones", [P, 2], F32, isOutput=False)
    out = nc.declare_dram_parameter("out", [SQ, D], F32, isOutput=True)

    with tile.TileContext(nc) as tc, ExitStack() as ctx:
        pool = ctx.enter_context(tc.tile_pool(name="main", bufs=1))
        ot = pool.tile([P, 2], F32)
        # d-chunk d at [:, d*S : (d+1)*S] (free axis = s over all 2048
        # keys); doubles as the scores lhsT in phase C.
        xT = pool.tile([P, ND * S], BF16, tag="xT")
        # e-chunk e at [:, e*SQ : (e+1)*SQ] (free axis = s query)
        yT = pool.tile([P, ND * SQ], BF16, tag="yT")
        # x natural: t-chunk t at [:, t*D : (t+1)*D]
        xc = pool.tile([P, NT * D], BF16, tag="xc")
        # A, e-major: e-block at [:, e*D : (e+1)*D]
        wt = pool.tile([P, ND * D], BF16, tag="w")

        exp_pool = ctx.enter_context(tc.tile_pool(name="exp", bufs=1))
        out_pool = ctx.enter_context(tc.tile_pool(name="outp", bufs=4))
        recip_pool = ctx.enter_context(tc.tile_pool(name="recip", bufs=4))
        partial_pool = ctx.enter_context(tc.tile_pool(name="partial", bufs=2))
        # one rotating pool for warmup/projection/scores groups + the
        # attn accumulators and the tiny denominator matmul
        ps_main = ctx.enter_context(tc.tile_pool(name="ps_main", bufs=4,
                                                 space="PSUM"))
        ps_av = ctx.enter_context(tc.tile_pool(name="ps_av", bufs=3,
                                               space="PSUM"))
        ps_sum = ctx.enter_context(tc.tile_pool(name="ps_sum", bufs=1,
                                                space="PSUM"))

        # PE warmup on uninitialized yT contents (yT's first real write
        # is a DVE eviction later, so no dependency and no delay);
        # garbage results land in a PSUM bank that is never read.
        ps_w = ps_main.tile([P, 512], F32, tag="ps")
        for i in range(N_WARMUP):
            nc.tensor.matmul(ps_w[:], yT[:, 0:P], yT[:, SQ:SQ + 512],
                             start=(i == 0), stop=(i == N_WARMUP - 1))

        # DMA issue order = queue assignment (round-robin) = arrival
        # order; chunks sized so no queue carries >96 rows before the
        # first matmul group's inputs are complete.
        for c in range(4):          # A e=0 in quarter-partition chunks
            nc.sync.dma_start(
                out=wt[c * 32:(c + 1) * 32, 0:D],
                in_=wa[c * 32:(c + 1) * 32, 0:D],
            )
        for d in range(ND):         # x^T s<1024 in half-partition chunks
            for h in range(2):
                nc.sync.dma_start(
                    out=xT[h * 64:(h + 1) * 64, d * S: d * S + SQ],
                    in_=xt[d * P + h * 64: d * P + (h + 1) * 64, 0:SQ],
                )
        for k in range(2, 2 * ND):  # A e=1..7 in half-partition chunks
            e, h = k // 2, k % 2
            nc.sync.dma_start(
                out=wt[h * 64:(h + 1) * 64, e * D:(e + 1) * D],
                in_=wa[h * 64:(h + 1) * 64, e * D:(e + 1) * D],
            )
        for d in range(ND):         # x^T s>=1024
            for h in range(2):
                nc.sync.dma_start(
                    out=xT[h * 64:(h + 1) * 64, d * S + SQ:(d + 1) * S],
                    in_=xt[d * P + h * 64: d * P + (h + 1) * 64, SQ:S],
                )
        for t in range(NT):         # x natural
            for h in range(2):
                nc.sync.dma_start(
                    out=xc[h * 64:(h + 1) * 64, t * D:(t + 1) * D],
                    in_=x[t * P + h * 64: t * P + h * 64 + 64, :],
                )
        nc.sync.dma_start(out=ot[:], in_=ones[:])

        def project(sh):
            # yT[:, e*SQ + sh*512 ...] for all e; e ascending matches
            # A's DMA arrival order. 512-wide stripes only — narrower
            # ones are LDWEIGHTS-bound.
            for e in range(ND):
                ps = ps_main.tile([P, 512], F32, tag="ps")
                for dd in range(ND):
                    nc.tensor.matmul(
                        ps[:],
                        wt[:, e * D + dd * P: e * D + (dd + 1) * P],
                        xT[:, dd * S + sh * 512: dd * S + sh * 512 + 512],
                        start=(dd == 0), stop=(dd == ND - 1),
                    )
                nc.vector.tensor_copy(
                    yT[:, e * SQ + sh * 512: e * SQ + sh * 512 + 512], ps[:],
                )

        def score_block(blk):
            # scoresT -> exp (ScalarE) with fp32 denominator partials
            # accumulating on the DVE behind the scores loop
            expT = exp_pool.tile([P, NT * SB], BF16, tag="expT")
            partial = partial_pool.tile([P, SB], F32, tag="partial")
            for t in range(NT):
                ps = ps_main.tile([P, SB], F32, tag="ps")
                for dd in range(ND):
                    nc.tensor.matmul(
                        ps[:],
                        xT[:, dd * S + t * P: dd * S + (t + 1) * P],
                        yT[:, dd * SQ + blk * SB: dd * SQ + (blk + 1) * SB],
                        start=(dd == 0), stop=(dd == ND - 1),
                    )
                nc.scalar.activation(expT[:, t * SB:(t + 1) * SB], ps[:], AFT.Exp)
                if t == 1:
                    nc.vector.tensor_add(
                        partial[:], expT[:, 0:SB], expT[:, SB:2 * SB])
                elif t >= 2:
                    nc.vector.tensor_add(
                        partial[:], partial[:], expT[:, t * SB:(t + 1) * SB])
            return expT, partial

        def attn_block(blk, expT, partial):
            for ss in range(SB // P):
                # first attn half-group goes ahead of the tiny
                # denominator matmul so the PE never waits on the DVE
                # partial chain; the reciprocal is ready by eviction.
                psa0 = ps_av.tile([P, 512], F32, tag="psa")
                for t in range(NT):
                    nc.tensor.matmul(
                        psa0[:],
                        expT[:, t * SB + ss * P: t * SB + (ss + 1) * P],
                        xc[:, t * D: t * D + 512],
                        start=(t == 0), stop=(t == NT - 1),
                    )
                pss = ps_sum.tile([P, 2], F32, tag="pss")
                nc.tensor.matmul(
                    pss[:], partial[:, ss * P:(ss + 1) * P], ot[:],
                    start=True, stop=True,
                )
                rec = recip_pool.tile([P, 1], F32, tag="rec")
                nc.vector.reciprocal(rec[:], pss[:, 0:1])
                psa1 = ps_av.tile([P, 512], F32, tag="psa")
                for t in range(NT):
                    nc.tensor.matmul(
                        psa1[:],
                        expT[:, t * SB + ss * P: t * SB + (ss + 1) * P],
                        xc[:, t * D + 512: t * D + 1024],
                        start=(t == 0), stop=(t == NT - 1),
                    )
                row0 = blk * SB + ss * P
                for dh, psa in ((0, psa0), (1, psa1)):
                    ob = out_pool.tile([P, 512], F32, tag="ob")
                    nc.vector.tensor_scalar_mul(ob[:], psa[:], rec[:, 0:1])
                    # two row-halves on two hw queues to shrink the
                    # final-store tail
                    for rh in range(2):
                        nc.sync.dma_start(
                            out=out[row0 + rh * 64:row0 + (rh + 1) * 64,
                                    dh * 512:dh * 512 + 512],
                            in_=ob[rh * 64:(rh + 1) * 64, :],
                        )

        project(0)
        expT, partial = score_block(0)
        attn_block(0, expT, partial)
        project(1)
        expT, partial = score_block(1)
        attn_block(1, expT, partial)

    nc.finalize()
    _NC_CACHE = nc
    return nc


def kernel(inputs, Wq, Wk):
    global LAST_RESULT
    x = np.asarray(inputs, dtype=np.float32)
    assert x.shape == (B_FULL, S_FULL, D_FULL)
    A = (np.asarray(Wq, dtype=np.float32) @ np.asarray(Wk, dtype=np.float32).T
         ) * np.float32(0.125)
    # walayout[p, e*D + dd*P + j] = A[dd*P + p, e*P + j]
    wa = np.ascontiguousarray(
        A.reshape(ND, P, ND, P).transpose(1, 2, 0, 3).reshape(P, ND * D)
        .astype(ml_dtypes.bfloat16))
    ones = np.ones((P, 2), dtype=np.float32)

    nc = _build_nc()

    in_maps = []
    for c in range(N_CORES):
        b, h = c // 2, c % 2
        xb = x[b]
        if h:
            xb = np.concatenate([xb[SQ:], xb[:SQ]], axis=0)
        xb16 = xb.astype(ml_dtypes.bfloat16)
        in_maps.append({
            "x": np.ascontiguousarray(xb16),
            "xt": np.ascontiguousarray(xb16.T),
            "wa": wa,
            "ones": ones,
        })

    kwargs = {}
    if TRACE:
        kwargs = {"trace": True, "tmpdir": TRACE_DIR}
    res = run_bass_kernel_spmd(nc, in_maps, list(range(N_CORES)), **kwargs)
    LAST_RESULT = res

    full = np.empty((B_FULL, S_FULL, D_FULL), dtype=np.float32)
    for c in range(N_CORES):
        b, h = c // 2, c % 2
        full[b, h * SQ:(h + 1) * SQ, :] = res.results[c]["out"]
    return full
